# revision 6
# baseline (speedup 1.0000x reference)
"""AttnDecoderRNN Trainium2 kernel (8 NeuronCores, data-parallel over batch).

Layout strategy:
  - Batch rows assigned rank-order: global row 8*j + c -> core c, slot j.
    lengths_enc is sorted descending, so slot j has chunk count
    K_j = ceil(lengths[8j]/128), identical across cores (SPMD-safe) and
    roughly balanced (the ragged tail is skipped entirely).
  - Streams (input_attW_enc / input_enc) processed in natural layout
    (128 timesteps on partitions, 512 features on free dim), one big DMA
    per (slot, tensor).
  - Per chunk: DVE add (attW_dec broadcast) -> ACT tanh -> DVE fused
    mul+reduce (scalar_tensor_tensor accum_out) -> ACT exp(+mask bias) ->
    one-hot select -> PE matmuls accumulate attn (8,512) and denom (8,1)
    in PSUM across the whole phase.
  - Small dense layers (prenet, 3 GRU cells with h0=0 => *_hh matmuls are
    dead, wdec/wsc/wout) run feature-major: out = lhsT.T @ rhs with
    lhsT = 128x128 weight tiles streamed from a host-packed array and
    rhs = activations (128, 8 rows). ctx part of ga_ih multiplies zeros
    and is pruned host-side.
"""
import sys
import numpy as np

for _p in ('/opt/trn_rl_repo',):
    if _p not in sys.path:
        sys.path.append(_p)

import concourse.bacc as bacc
import concourse.mybir as mybir
from concourse.tile import TileContext
from concourse.bass_utils import run_bass_kernel_spmd

F32 = mybir.dt.float32
AF = mybir.ActivationFunctionType
ALU = mybir.AluOpType

N, T, H, O, S, R = 64, 1024, 256, 80, 128, 2
H2 = 2 * H            # 512
CH = 128              # timestep chunk
D = 512               # feature width of streams
NC_ = 8               # cores
J = 8                 # slots per core
G = 32                # weight tiles per staging slab
NEG = -30000.0        # additive mask (exp -> exact 0 in f32)

_BUILD_CACHE = {}
LAST_RESULTS = None


# --------------------------------------------------------------------------
# host-side packing
# --------------------------------------------------------------------------

def _tiles_mk(wT, m, k):
    """lhsT tile (K=128, M=128) for out-chunk m, in-chunk k of weight W
    (out_f, in_f): tile = W[m*128:(m+1)*128, k*128:(k+1)*128].T zero-padded."""
    t = np.zeros((128, 128), np.float32)
    blk = wT[m * 128:(m + 1) * 128, k * 128:(k + 1) * 128]
    t[:blk.shape[1], :blk.shape[0]] = blk.T
    return t


def _pack_weights(inp):
    f32 = lambda k: np.asarray(inp[k], np.float32)
    ga_ih = f32("ga_ih")
    ga_eff = np.concatenate([ga_ih[:, 0:H], ga_ih[:, 3 * H:3 * H + S]], axis=1)
    specs = [
        (f32("pw1"), 4, 1),      # (512,80)
        (f32("pw2"), 2, 4),      # (256,512)
        (ga_eff, 12, 3),         # (1536,384)
        (f32("wdec"), 4, 4),     # (512,512)
        (f32("wsc"), 4, 9),      # (512,1152)
        (f32("g1_ih"), 12, 9),   # (1536,1152)
        (f32("g2_ih"), 12, 4),   # (1536,512)
        (f32("wout"), 2, 4),     # (160,512)
    ]
    tiles = []
    for w, ms, ks in specs:
        for m in range(ms):
            for k in range(ks):
                tiles.append(_tiles_mk(w, m, k))
    return np.stack(tiles)  # (NT, 128, 128)


def _col(v, lo, hi):
    c = np.zeros(128, np.float32)
    seg = v[lo:hi]
    c[:len(seg)] = seg
    return c


def _pack_biases(inp):
    f32 = lambda k: np.asarray(inp[k], np.float32)
    cols = []

    def add4(v, base=0, n=4):
        for m in range(n):
            cols.append(_col(v, base + m * 128, base + (m + 1) * 128))

    add4(f32("pb1"))                                     # 0..3
    add4(f32("pb2"), n=2)                                # 4..5
    for name in ("ga", "g1", "g2"):
        bih, bhh = f32(name + "_bih"), f32(name + "_bhh")
        add4(bih + bhh, 0)                               # r combined
        add4(bih + bhh, H2)                              # z combined
        add4(bhh, 2 * H2)                                # bhh_n
        add4(bih, 2 * H2)                                # bih_n
    add4(f32("bdec"))
    add4(f32("bsc"))
    add4(f32("bout"), n=2)
    return np.stack(cols, axis=1)                        # (128, NBC)


# bias column index map (must match _pack_biases order)
BIX = {}
_i = 0
BIX["pb1"] = 0; _i = 4
BIX["pb2"] = _i; _i += 2
for _nm in ("ga", "g1", "g2"):
    BIX[_nm + "_r"] = _i; _i += 4
    BIX[_nm + "_z"] = _i; _i += 4
    BIX[_nm + "_bhhn"] = _i; _i += 4
    BIX[_nm + "_bihn"] = _i; _i += 4
BIX["bdec"] = _i; _i += 4
BIX["bsc"] = _i; _i += 4
BIX["bout"] = _i; _i += 2
NBC = _i


# --------------------------------------------------------------------------
# device program
# --------------------------------------------------------------------------

class _WStream:
    """Sequential weight-tile reader with slab staging DMA."""

    def __init__(self, nc, pool, wpack_d, nt):
        self.nc, self.pool, self.d, self.nt = nc, pool, wpack_d, nt
        self.i = 0
        self.slab = None

    def next(self):
        s, off = divmod(self.i, G)
        if off == 0:
            n = min(G, self.nt - s * G)
            slab = self.pool.tile([128, G * 128], F32, tag="w", name=f"wslab{s}")
            self.nc.sync.dma_start(
                slab[:, 0:n * 128].rearrange("p (t q) -> p t q", t=n),
                self.d[s * G:s * G + n].rearrange("t k q -> k t q"))
            self.slab = slab
        ap = self.slab[:, off * 128:(off + 1) * 128]
        self.i += 1
        return ap


def _build(Kjs):
    nc = bacc.Bacc("TRN2", target_bir_lowering=False, debug=False,
                   num_devices=NC_)
    NT = 4 + 8 + 36 + 16 + 36 + 108 + 48 + 8  # 264

    attw_d = nc.dram_tensor("attw", (J, T, D), F32, kind="ExternalInput")
    enc_d = nc.dram_tensor("enc", (J, T, D), F32, kind="ExternalInput")
    xdec_d = nc.dram_tensor("xdec", (128, J), F32, kind="ExternalInput")
    sv_d = nc.dram_tensor("sv", (128, J), F32, kind="ExternalInput")
    maskb_d = nc.dram_tensor("maskb", (128, J * 8), F32, kind="ExternalInput")
    onehots_d = nc.dram_tensor("onehots", (128, J * 8), F32, kind="ExternalInput")
    wpack_d = nc.dram_tensor("wpack", (NT, 128, 128), F32, kind="ExternalInput")
    bpack_d = nc.dram_tensor("bpack", (128, NBC), F32, kind="ExternalInput")
    wattnb_d = nc.dram_tensor("wattnb", (128, D), F32, kind="ExternalInput")
    ident_d = nc.dram_tensor("ident", (128, 128), F32, kind="ExternalInput")
    selbc_d = nc.dram_tensor("selbc", (8, J * 128), F32, kind="ExternalInput")
    ones_d = nc.dram_tensor("onescol", (128, 1), F32, kind="ExternalInput")
    outo_d = nc.dram_tensor("out_o", (J, R * O), F32, kind="ExternalOutput")
    outa_d = nc.dram_tensor("out_attn", (J, D), F32, kind="ExternalOutput")

    with TileContext(nc) as tc:
        with tc.tile_pool(name="consts", bufs=1) as consts, \
             tc.tile_pool(name="wstage", bufs=4) as wstage, \
             tc.tile_pool(name="acts", bufs=1) as acts, \
             tc.tile_pool(name="sg", bufs=2) as sg, \
             tc.tile_pool(name="stream", bufs=2) as stream, \
             tc.tile_pool(name="chunk", bufs=3) as chunkp, \
             tc.tile_pool(name="scrp", bufs=1) as scrp, \
             tc.tile_pool(name="ps_g", bufs=3, space="PSUM") as ps_g, \
             tc.tile_pool(name="ps_b", bufs=1, space="PSUM") as ps_b, \
             tc.tile_pool(name="ps_bc", bufs=1, space="PSUM") as ps_bc, \
             tc.tile_pool(name="ps_tr", bufs=2, space="PSUM") as ps_tr:

            # ---- constants
            xdec = consts.tile([128, J], F32)
            sv = consts.tile([128, J], F32)
            maskb = consts.tile([128, J * 8], F32)
            onehots = consts.tile([128, J * 8], F32)
            bpack = consts.tile([128, NBC], F32)
            wattnb = consts.tile([128, D], F32)
            ident = consts.tile([128, 128], F32)
            selbc = consts.tile([8, J * 128], F32)
            onescol = consts.tile([128, 1], F32)
            for t_, d_ in ((xdec, xdec_d), (sv, sv_d), (maskb, maskb_d),
                           (onehots, onehots_d), (bpack, bpack_d),
                           (wattnb, wattnb_d), (ident, ident_d),
                           (selbc, selbc_d), (onescol, ones_d)):
                nc.sync.dma_start(t_[:], d_[:])

            def bvec(name, m):
                i = BIX[name] + m
                return bpack[:, i:i + 1]

            ws = _WStream(nc, wstage, wpack_d, NT)

            def dense(rhs_list, m_chunks, name):
                """out_m (PSUM (128,J)) for m in m_chunks; yields psum tiles."""
                outs = []
                for m in range(m_chunks):
                    ps = ps_g.tile([128, J], F32, tag="g", name=f"ps_{name}{m}")
                    nk = len(rhs_list)
                    for ki, rhs in enumerate(rhs_list):
                        nc.tensor.matmul(ps[:], ws.next(), rhs,
                                         start=(ki == 0), stop=(ki == nk - 1))
                    outs.append(ps)
                return outs

            def act_store(ps, func, bias, name):
                t_ = acts.tile([128, J], F32, name=name)
                nc.scalar.activation(t_[:], ps[:], func, bias=bias)
                return t_

            def gru(rhs_list, pfx):
                """3-gate GRU with h0=0. Returns 4 h-chunks (128,J) SBUF."""
                rs, zs, hs = [], [], []
                for m in range(12):
                    ps = ps_g.tile([128, J], F32, tag="g", name=f"ps_{pfx}{m}")
                    nk = len(rhs_list)
                    for ki, rhs in enumerate(rhs_list):
                        nc.tensor.matmul(ps[:], ws.next(), rhs,
                                         start=(ki == 0), stop=(ki == nk - 1))
                    if m < 4:
                        rs.append(act_store(ps, AF.Sigmoid, bvec(pfx + "_r", m),
                                            f"{pfx}r{m}"))
                    elif m < 8:
                        zs.append(act_store(ps, AF.Sigmoid, bvec(pfx + "_z", m - 4),
                                            f"{pfx}z{m - 4}"))
                    else:
                        mm = m - 8
                        rb = sg.tile([128, J], F32, tag="rb", name=f"{pfx}rb{mm}")
                        nc.vector.tensor_scalar_mul(rb[:], rs[mm][:],
                                                    bvec(pfx + "_bhhn", mm))
                        s_ = sg.tile([128, J], F32, tag="s", name=f"{pfx}s{mm}")
                        nc.vector.tensor_tensor(s_[:], ps[:], rb[:], op=ALU.add)
                        n_ = sg.tile([128, J], F32, tag="n", name=f"{pfx}n{mm}")
                        nc.scalar.activation(n_[:], s_[:], AF.Tanh,
                                             bias=bvec(pfx + "_bihn", mm))
                        zn = sg.tile([128, J], F32, tag="zn", name=f"{pfx}zn{mm}")
                        nc.vector.tensor_tensor(zn[:], zs[mm][:], n_[:],
                                                op=ALU.mult)
                        h_ = acts.tile([128, J], F32, name=f"{pfx}h{mm}")
                        nc.vector.tensor_tensor(h_[:], n_[:], zn[:],
                                                op=ALU.subtract)
                        hs.append(h_)
                return hs

            # ================= phase A =================
            pre1 = [act_store(ps, AF.Relu, bvec("pb1", m), f"pre1_{m}")
                    for m, ps in enumerate(dense([xdec[:]], 4, "p1"))]
            pre2 = [act_store(ps, AF.Relu, bvec("pb2", m), f"pre2_{m}")
                    for m, ps in enumerate(dense([p[:] for p in pre1], 2, "p2"))]
            out_att = gru([pre2[0][:], pre2[1][:], sv[:]], "ga")
            awd = [act_store(ps, AF.Identity, bvec("bdec", m), f"awd{m}")
                   for m, ps in enumerate(
                       dense([h[:] for h in out_att], 4, "wd"))]

            # attW_dec rows (8, 512): transpose the 4 feature chunks
            awrows = acts.tile([8, D], F32)
            for g_ in range(4):
                pt = ps_tr.tile([8, 128], F32, tag="tr", name=f"awT{g_}")
                nc.tensor.matmul(pt[:], awd[g_][:, 0:8], ident[:],
                                 start=True, stop=True)
                nc.scalar.copy(awrows[:, g_ * 128:(g_ + 1) * 128], pt[:])

            # broadcast each row j across 128 partitions: one-hot K=8 matmul
            bcast = acts.tile([128, J * D], F32)
            for j in range(J):
                pbc = ps_bc.tile([128, D], F32, tag="bc", name=f"pbc{j}")
                nc.tensor.matmul(pbc[:], selbc[:, j * 128:(j + 1) * 128],
                                 awrows[:], start=True, stop=True)
                nc.scalar.copy(bcast[:, j * D:(j + 1) * D], pbc[:])

            # ================= phase B =================
            ps_attn = ps_b.tile([8, D], F32, name="ps_attn")
            ps_den = ps_b.tile([8, 1], F32, name="ps_den")
            first = True
            for j in range(J):
                Kj = Kjs[j]
                ar = stream.tile([128, 8 * D], F32, tag="ar", name=f"ar{j}")
                nc.sync.dma_start(
                    ar[:, 0:Kj * D].rearrange("p (c d) -> p c d", c=Kj),
                    attw_d[j].rearrange("(c p) d -> p c d", p=128)[:, 0:Kj, :])
                er = stream.tile([128, 8 * D], F32, tag="er", name=f"er{j}")
                nc.sync.dma_start(
                    er[:, 0:Kj * D].rearrange("p (c d) -> p c d", c=Kj),
                    enc_d[j].rearrange("(c p) d -> p c d", p=128)[:, 0:Kj, :])
                for c in range(Kj):
                    t1 = chunkp.tile([128, D], F32, tag="t1", name=f"t1_{j}_{c}")
                    nc.vector.tensor_tensor(
                        t1[:], ar[:, c * D:(c + 1) * D],
                        bcast[:, j * D:(j + 1) * D], op=ALU.add)
                    e_ = chunkp.tile([128, D], F32, tag="e", name=f"e_{j}_{c}")
                    nc.scalar.activation(e_[:], t1[:], AF.Tanh)
                    scr = scrp.tile([128, D], F32, tag="scr",
                                     name=f"scr_{j}_{c}")
                    score = chunkp.tile([128, 1], F32, tag="sc",
                                        name=f"sc_{j}_{c}")
                    nc.vector.scalar_tensor_tensor(
                        out=scr[:], in0=e_[:], scalar=0.0, in1=wattnb[:],
                        op0=ALU.bypass, op1=ALU.mult, accum_out=score[:])
                    wcol = chunkp.tile([128, 1], F32, tag="wc",
                                       name=f"wc_{j}_{c}")
                    nc.scalar.activation(wcol[:], score[:], AF.Exp,
                                         bias=maskb[:, j * 8 + c:j * 8 + c + 1])
                    wsel = chunkp.tile([128, 8], F32, tag="ws",
                                       name=f"ws_{j}_{c}")
                    nc.vector.tensor_scalar_mul(
                        wsel[:], onehots[:, j * 8:(j + 1) * 8], wcol[:, 0:1])
                    last = (j == J - 1 and c == Kj - 1)
                    nc.tensor.matmul(ps_den[:], wsel[:], onescol[:],
                                     start=first, stop=last)
                    nc.tensor.matmul(ps_attn[:], wsel[:],
                                     er[:, c * D:(c + 1) * D],
                                     start=first, stop=last)
                    first = False

            # normalize attn
            dmax = sg.tile([8, 1], F32, tag="dn", name="dmax")
            nc.vector.tensor_scalar_max(dmax[:], ps_den[:], 1e-12)
            recip = sg.tile([8, 1], F32, tag="dn2", name="recip")
            nc.vector.reciprocal(recip[:], dmax[:])
            attn_u = sg.tile([8, D], F32, tag="au", name="attn_u")
            nc.vector.tensor_copy(attn_u[:], ps_attn[:])
            attn_s = acts.tile([8, D], F32)
            nc.vector.tensor_scalar_mul(attn_s[:], attn_u[:], recip[:, 0:1])
            nc.sync.dma_start(outa_d[:], attn_s[:])

            # transpose attn rows -> feature-major chunks (128, 8)
            attnT = []
            for g_ in range(4):
                pt = ps_tr.tile([128, 8], F32, tag="tr", name=f"aT{g_}")
                nc.tensor.matmul(pt[:], attn_s[:, g_ * 128:(g_ + 1) * 128],
                                 ident[0:8, 0:8], start=True, stop=True)
                t_ = acts.tile([128, J], F32, name=f"attnT{g_}")
                nc.scalar.copy(t_[:], pt[:])
                attnT.append(t_)

            # ================= phase C =================
            odec = [a[:] for a in attnT] + [h[:] for h in out_att] + [sv[:]]
            res = [act_store(ps, AF.Identity, bvec("bsc", m), f"res{m}")
                   for m, ps in enumerate(dense(odec, 4, "sc"))]
            h1 = gru(odec, "g1")
            res2 = []
            for m in range(4):
                t_ = acts.tile([128, J], F32, name=f"res2_{m}")
                nc.vector.tensor_tensor(t_[:], res[m][:], h1[m][:], op=ALU.add)
                res2.append(t_)
            h2 = gru([r_[:] for r_ in res2], "g2")
            res3 = []
            for m in range(4):
                t_ = acts.tile([128, J], F32, name=f"res3_{m}")
                nc.vector.tensor_tensor(t_[:], res2[m][:], h2[m][:], op=ALU.add)
                res3.append(t_)
            outT = [act_store(ps, AF.Identity, bvec("bout", m), f"outT{m}")
                    for m, ps in enumerate(dense([r_[:] for r_ in res3], 2,
                                                 "wo"))]
            # transpose (160, 8) -> (8, 160)
            orows = acts.tile([8, R * O], F32)
            pt = ps_tr.tile([8, 128], F32, tag="tr", name="oT0")
            nc.tensor.matmul(pt[:], outT[0][:, 0:8], ident[:],
                             start=True, stop=True)
            nc.scalar.copy(orows[:, 0:128], pt[:])
            pt2 = ps_tr.tile([8, 32], F32, tag="tr", name="oT1")
            nc.tensor.matmul(pt2[:], outT[1][0:32, 0:8], ident[0:32, 0:32],
                             start=True, stop=True)
            nc.scalar.copy(orows[:, 128:160], pt2[:])
            nc.sync.dma_start(outo_d[:], orows[:])

    nc.compile()
    return nc


# --------------------------------------------------------------------------
# entry point
# --------------------------------------------------------------------------

def kernel(**inputs):
    lengths = np.asarray(inputs["lengths_enc"]).astype(np.int64)
    Kjs = tuple(int(np.ceil(max(int(lengths[8 * j:8 * j + 8].max()), 1) / CH))
                for j in range(J))

    if Kjs not in _BUILD_CACHE:
        _BUILD_CACHE[Kjs] = _build(Kjs)
    nc = _BUILD_CACHE[Kjs]

    f32 = lambda k: np.ascontiguousarray(np.asarray(inputs[k], np.float32))
    attw_full, enc_full = f32("input_attW_enc"), f32("input_enc")
    dec, svf = f32("input_dec"), f32("style_vec")[:, 0, :]
    battn = float(np.asarray(inputs["battn"], np.float32)[0])

    wpack = _pack_weights(inputs)
    bpack = _pack_biases(inputs)
    wattnb = np.ascontiguousarray(
        np.broadcast_to(np.asarray(inputs["wattn"], np.float32)[0:1, :],
                        (128, D)))
    ident = np.eye(128, dtype=np.float32)
    onescol = np.ones((128, 1), np.float32)
    onehots = np.zeros((128, J * 8), np.float32)
    for j in range(J):
        onehots[:, j * 8 + j] = 1.0
    selbc = np.zeros((8, J * 128), np.float32)
    for j in range(J):
        selbc[j, j * 128:(j + 1) * 128] = 1.0

    in_maps = []
    for c in range(NC_):
        rows = [8 * j + c for j in range(J)]
        xdec = np.zeros((128, J), np.float32)
        xdec[0:O, :] = dec[rows].T
        sv_c = np.ascontiguousarray(svf[rows].T)  # (128, 8)
        maskb = np.full((128, J * 8), battn + NEG, np.float32)
        for j in range(J):
            ln = int(lengths[rows[j]])
            for ch in range(Kjs[j]):
                t0 = ch * CH
                valid = np.clip(ln - t0, 0, CH)
                maskb[0:valid, j * 8 + ch] = battn
        in_maps.append(dict(
            attw=np.ascontiguousarray(attw_full[rows]),
            enc=np.ascontiguousarray(enc_full[rows]),
            xdec=xdec, sv=sv_c, maskb=maskb, onehots=onehots,
            wpack=wpack, bpack=bpack, wattnb=wattnb, ident=ident,
            selbc=selbc, onescol=onescol))

    import os as _os
    global LAST_RESULTS
    if _os.environ.get("KERNEL_SIM", "0") == "1":
        from types import SimpleNamespace
        from concourse.bass_interp import CoreSim
        results = []
        for c in range(NC_):
            sim = CoreSim(nc, trace=False)
            for k, v in in_maps[c].items():
                sim.tensor(k)[:] = v
            sim.simulate()
            results.append({k: np.array(sim.tensor(k))
                            for k in ("out_o", "out_attn")})
            print(f"sim core {c} done", flush=True)
        res = SimpleNamespace(results=results, exec_time_ns=None)
    else:
        _tr = _os.environ.get("KERNEL_TRACE", "0") == "1"
        res = run_bass_kernel_spmd(nc, in_maps, core_ids=list(range(NC_)),
                                   trace=_tr)
    LAST_RESULTS = res

    output = np.zeros((N, R, O), np.float32)
    attn = np.zeros((N, 1, D), np.float32)
    for c in range(NC_):
        r = res.results[c]
        for j in range(J):
            n = 8 * j + c
            output[n] = r["out_o"][j].reshape(R, O)
            attn[n, 0] = r["out_attn"][j]
    return output, attn
